# revision 22
# baseline (speedup 1.0000x reference)
import sys

sys.path.insert(0, "/opt/trn_rl_repo")

from contextlib import ExitStack

import ml_dtypes
import numpy as np

import concourse.bass as bass
import concourse.mybir as mybir
import concourse.tile as tile
from concourse import bacc, bass_utils
from concourse.bass_isa import ReduceOp

N, OBS, ENC, ACT, K = 16384, 512, 512, 64, 8
ALPHA = 1.0
NCORES = 8
R = N // NCORES  # rows per core
P = 128
NT = R // P  # n-tiles per core
NB = R // 512  # 512-wide n blocks
NH = ENC // P
NO = OBS // P
F32 = mybir.dt.float32
AX = mybir.AluOpType


def build_nc(mm_dtype=F32):
    # Bacc (not bass.Bass): its finalize() runs move_matmul_waits_to_ldweights
    # + generate_event_semaphores, required by TRN2's 1-wait-per-inst limit.
    nc = bacc.Bacc("TRN2", target_bir_lowering=False)
    x0t = nc.declare_dram_parameter("x0t", [OBS, R], mm_dtype, isOutput=False)
    x1t = nc.declare_dram_parameter("x1t", [OBS, R], mm_dtype, isOutput=False)
    ut = nc.declare_dram_parameter("ut", [ACT, R], mm_dtype, isOutput=False)
    wet = nc.declare_dram_parameter("wet", [OBS, ENC], mm_dtype, isOutput=False)
    at = nc.declare_dram_parameter("at", [K, ENC, ENC], mm_dtype, isOutput=False)
    ball = nc.declare_dram_parameter("ball", [K, ACT, ENC], mm_dtype, isOutput=False)
    cwt = nc.declare_dram_parameter("cwt", [ENC, K], mm_dtype, isOutput=False)
    cb = nc.declare_dram_parameter("cb", [1, K], F32, isOutput=False)
    loss = nc.declare_dram_parameter("loss_out", [1, 1], F32, isOutput=True)

    with tile.TileContext(nc) as tc, ExitStack() as ctx:
        const = ctx.enter_context(tc.tile_pool(name="const", bufs=1))
        stream = ctx.enter_context(tc.tile_pool(name="stream", bufs=2))
        dwork = ctx.enter_context(tc.tile_pool(name="dwork", bufs=3))
        psumA = ctx.enter_context(tc.tile_pool(name="psumA", bufs=4, space="PSUM"))
        psumS = ctx.enter_context(tc.tile_pool(name="psumS", bufs=2, space="PSUM"))

        # --- resident weights/activations ---
        wet_sb = const.tile([P, NO, ENC], mm_dtype)  # [o%128, o//128, h]
        nc.sync.dma_start(wet_sb[:], wet.rearrange("(c p) h -> p c h", p=P))
        ball_sb = const.tile([ACT, K, ENC], mm_dtype)  # [a, k, e]
        nc.sync.dma_start(ball_sb[:], ball.rearrange("k a e -> a k e"))
        cwt_sb = const.tile([P, NH, K], mm_dtype)  # [h%128, h//128, k]
        nc.sync.dma_start(cwt_sb[:], cwt.rearrange("(c p) k -> p c k", p=P))
        cb128 = const.tile([P, K], F32)
        nc.sync.dma_start(cb128[:], bass.AP(tensor=cb, offset=0, ap=[[0, P], [1, K]]))
        ut_sb = const.tile([ACT, R], mm_dtype)  # [a, n]
        nc.sync.dma_start(ut_sb[:], ut[:])

        x0et = const.tile([P, NH, R], mm_dtype)  # [h%128, h//128, n]
        x1e = const.tile([P, NT, ENC], F32)  # [n%128, n//128, e]

        iota_i = const.tile([P, K], mybir.dt.int32)
        nc.gpsimd.iota(iota_i[:], pattern=[[1, K]], base=0, channel_multiplier=0)
        iota_f = const.tile([P, K], F32)
        nc.scalar.copy(iota_f[:], iota_i[:])
        oh_all = const.tile([P, NT * K], F32)
        sq_all = const.tile([P, NT * K], F32)
        acc = const.tile([P, NT], F32)

        x0t_r = x0t.rearrange("(c p) n -> p c n", p=P)
        x1t_r = x1t.rearrange("(c p) n -> p c n", p=P)
        at_r = at.rearrange("k (c p) e -> p k c e", p=P)

        # --- phase A: encode (X0e^T and X1e) ---
        for nb in range(NB):
            ns = slice(nb * 512, (nb + 1) * 512)
            x0c = stream.tile([P, NO, 512], mm_dtype, name="x0c")
            nc.sync.dma_start(x0c[:], x0t_r[:, :, ns])
            x1c = stream.tile([P, NO, 512], mm_dtype, name="x1c")
            nc.sync.dma_start(x1c[:], x1t_r[:, :, ns])
            for hc in range(NH):
                pt = psumA.tile([P, 512], F32, name="pA")
                for oc in range(NO):
                    nc.tensor.matmul(
                        pt[:],
                        wet_sb[:, oc, hc * P : (hc + 1) * P],
                        x0c[:, oc, :],
                        start=(oc == 0),
                        stop=(oc == NO - 1),
                    )
                nc.scalar.copy(x0et[:, hc, ns], pt[:])
            for j in range(4):
                nt = nb * 4 + j
                pt = psumA.tile([P, 512], F32, name="pA")
                for oc in range(NO):
                    nc.tensor.matmul(
                        pt[:],
                        x1c[:, oc, j * P : (j + 1) * P],
                        wet_sb[:, oc, :],
                        start=(oc == 0),
                        stop=(oc == NO - 1),
                    )
                nc.scalar.copy(x1e[:, nt, :], pt[:])

        # --- phase B: router logits, argmax, one-hot ---
        for nt in range(NT):
            nts = slice(nt * P, (nt + 1) * P)
            pl = psumS.tile([P, K], F32, name="pl")
            for hc in range(NH):
                nc.tensor.matmul(
                    pl[:],
                    x0et[:, hc, nts],
                    cwt_sb[:, hc, :],
                    start=(hc == 0),
                    stop=(hc == NH - 1),
                )
            lg = dwork.tile([P, K], F32, name="lg")
            nc.vector.tensor_tensor(lg[:], pl[:], cb128[:], AX.add)
            mx = dwork.tile([P, K], F32, name="mx")
            ix = dwork.tile([P, K], mybir.dt.uint32, name="ix")
            nc.vector.max_with_indices(mx[:], ix[:], lg[:])
            ixf = dwork.tile([P, 1], F32, name="ixf")
            nc.scalar.copy(ixf[:], ix[:, 0:1])
            nc.vector.tensor_scalar(
                oh_all[:, nt * K : (nt + 1) * K],
                iota_f[:],
                ixf[:],
                None,
                op0=AX.is_equal,
            )

        # --- phase C: per-expert preds, squared error ---
        for k in range(K):
            atk = stream.tile([P, NH, ENC], mm_dtype, name="atk")
            nc.sync.dma_start(atk[:], at_r[:, k, :, :])
            for nt in range(NT):
                nts = slice(nt * P, (nt + 1) * P)
                pd = psumA.tile([P, 512], F32, name="pA")
                for hc in range(NH):
                    nc.tensor.matmul(
                        pd[:],
                        x0et[:, hc, nts],
                        atk[:, hc, :],
                        start=(hc == 0),
                        stop=False,
                    )
                nc.tensor.matmul(
                    pd[:], ut_sb[:, nts], ball_sb[:, k, :], start=False, stop=True
                )
                # GPSIMD cannot read PSUM and TensorScalarPtr is illegal on Pool:
                # vector does the subtract (PSUM->SBUF), ACT does square+accum.
                df = dwork.tile([P, ENC], F32, name="df")
                nc.vector.tensor_tensor(df[:], x1e[:, nt, :], pd[:], AX.subtract)
                sj = dwork.tile([P, ENC], F32, name="sj")
                nc.scalar.activation(
                    sj[:],
                    df[:],
                    mybir.ActivationFunctionType.Square,
                    accum_out=sq_all[:, nt * K + k : nt * K + k + 1],
                )

        # --- phase D: select routed expert's sq, accumulate ---
        for nt in range(NT):
            ks = slice(nt * K, (nt + 1) * K)
            sel = dwork.tile([P, K], F32, name="sel")
            nc.vector.scalar_tensor_tensor(
                sel[:],
                sq_all[:, ks],
                1.0,
                oh_all[:, ks],
                op0=AX.mult,
                op1=AX.mult,
                accum_out=acc[:, nt : nt + 1],
            )

        out_sb = const.tile([1, 1], F32)
        nc.gpsimd.tensor_reduce(
            out_sb[:], acc[:], axis=mybir.AxisListType.XYZWC, op=AX.add
        )
        nc.sync.dma_start(loss[:], out_sb[:])

    nc.finalize()
    return nc


T_TILES = 136  # 17408 padded slots: 16384 rows + <=127 pad/expert + global pad
NT2 = T_TILES // NCORES  # 17 tiles per core
R2 = NT2 * P  # 2176 rows per core
CHUNKS = [(0, 512), (512, 512), (1024, 512), (1536, 512), (2048, 128)]


def build_nc_routed(mm_dtype):
    # Encoder folded into weights on host: pred - x1e =
    #   x0 @ (W^T A_k^T) + u @ B_k - x1 @ W^T  -> 9 matmuls into one PSUM bank,
    # ACT squares straight from PSUM. Square kills the sign, so wetn = -W^T.
    nc = bacc.Bacc("TRN2", target_bir_lowering=False)
    x0t = nc.declare_dram_parameter("x0t", [OBS, R2], mm_dtype, isOutput=False)
    x1t = nc.declare_dram_parameter("x1t", [OBS, R2], mm_dtype, isOutput=False)
    ut = nc.declare_dram_parameter("ut", [ACT, R2], mm_dtype, isOutput=False)
    wetn = nc.declare_dram_parameter("wetn", [OBS, ENC], mm_dtype, isOutput=False)
    atb = nc.declare_dram_parameter("atb", [P, NT2 * NO, ENC], mm_dtype, isOutput=False)
    ballb = nc.declare_dram_parameter("ballb", [ACT, NT2, ENC], mm_dtype, isOutput=False)
    loss = nc.declare_dram_parameter("loss_out", [1, 1], F32, isOutput=True)

    with tile.TileContext(nc) as tc, ExitStack() as ctx:
        const = ctx.enter_context(tc.tile_pool(name="const", bufs=1))
        stream = ctx.enter_context(tc.tile_pool(name="stream", bufs=6))
        dwork = ctx.enter_context(tc.tile_pool(name="dwork", bufs=3))
        psumA = ctx.enter_context(tc.tile_pool(name="psumA", bufs=4, space="PSUM"))

        wetn_sb = const.tile([P, NO, ENC], mm_dtype)
        nc.sync.dma_start(wetn_sb[:], wetn.rearrange("(c p) h -> p c h", p=P))
        ut_sb = const.tile([ACT, R2], mm_dtype)
        nc.sync.dma_start(ut_sb[:], ut[:])
        ballb_sb = const.tile([ACT, NT2, ENC], mm_dtype)
        nc.sync.dma_start(ballb_sb[:], ballb[:])

        x0t_r = x0t.rearrange("(c p) n -> p c n", p=P)
        x1t_r = x1t.rearrange("(c p) n -> p c n", p=P)
        x0t_sb = const.tile([P, NO, R2], mm_dtype)
        x1t_sb = const.tile([P, NO, R2], mm_dtype)
        H = R2 // 4
        for h in range(4):
            hs = slice(h * H, (h + 1) * H)
            nc.sync.dma_start(x0t_sb[:, :, hs], x0t_r[:, :, hs])
            nc.sync.dma_start(x1t_sb[:, :, hs], x1t_r[:, :, hs])

        acc = const.tile([P, NT2], F32)

        for nt in range(NT2):
            nts = slice(nt * P, (nt + 1) * P)
            atk = stream.tile([P, NO, ENC], mm_dtype, name="atk")
            nc.sync.dma_start(atk[:], atb[:, nt * NO : (nt + 1) * NO, :])
            pd = psumA.tile([P, ENC], F32, name="pA")
            for oc in range(NO):
                nc.tensor.matmul(
                    pd[:],
                    x0t_sb[:, oc, nts],
                    atk[:, oc, :],
                    start=(oc == 0),
                    stop=False,
                )
            nc.tensor.matmul(
                pd[:],
                ut_sb[:, nts],
                ballb_sb[:, nt, :],
                start=False,
                stop=False,
            )
            for oc in range(NO):
                nc.tensor.matmul(
                    pd[:],
                    x1t_sb[:, oc, nts],
                    wetn_sb[:, oc, :],
                    start=False,
                    stop=(oc == NO - 1),
                )
            sj = dwork.tile([P, ENC], F32, name="sj")
            nc.scalar.activation(
                sj[:],
                pd[:],
                mybir.ActivationFunctionType.Square,
                accum_out=acc[:, nt : nt + 1],
            )

        red = const.tile([P, 1], F32)
        nc.vector.tensor_reduce(red[:], acc[:], axis=mybir.AxisListType.XYZW, op=AX.add)
        nc.gpsimd.partition_all_reduce(red[:], red[:], P, ReduceOp.add)
        nc.sync.dma_start(loss[:], red[0:1, 0:1])

    nc.finalize()
    return nc


_NC_CACHE = {}
MM_BF16 = True
ROUTED = True


def _get_nc():
    key = ("routed" if ROUTED else "dense", MM_BF16)
    if key not in _NC_CACHE:
        # bf16: 1 cyc/row on PE (f32r measured ~2 due to 4B SBUF moving-read cap)
        dt = mybir.dt.bfloat16 if MM_BF16 else mybir.dt.float32r
        _NC_CACHE[key] = build_nc_routed(dt) if ROUTED else build_nc(dt)
    return _NC_CACHE[key]


def _route_slots(X0, W_enc, C_w, C_b):
    # f64 router on host: argmax(X0 @ W_enc.T @ C_w.T + C_b) per row
    m = (C_w.astype(np.float64) @ W_enc.astype(np.float64)).T  # [OBS, K]
    logits = X0.astype(np.float64) @ m + C_b.astype(np.float64)
    inds = np.argmax(logits, axis=1)
    rows_l, eids = [], []
    for k in range(K):
        rk = np.nonzero(inds == k)[0]
        pad = (-len(rk)) % P
        rows_l.append(rk)
        rows_l.append(np.full(pad, -1, np.int64))
        eids += [k] * ((len(rk) + pad) // P)
    rows = np.concatenate(rows_l)
    rows = np.concatenate([rows, np.full(T_TILES * P - len(rows), -1, np.int64)])
    eids += [0] * (T_TILES - len(eids))
    return rows, np.asarray(eids)


def make_in_maps(X1, X0, U, W_enc, A_all, B_rest, C_w, C_b):
    mm_np = ml_dtypes.bfloat16 if MM_BF16 else np.float32
    wet = np.ascontiguousarray(W_enc.T).astype(mm_np)  # [OBS, ENC]
    at = A_all.transpose(0, 2, 1).astype(mm_np)  # [K, h, e]
    b0 = np.eye(ENC, dtype=np.float32)[:ACT]
    ball = np.concatenate([b0[None], B_rest], axis=0).astype(mm_np)  # [K, a, e]

    if not ROUTED:
        cwt = np.ascontiguousarray(C_w.T).astype(mm_np)
        cb = np.ascontiguousarray(C_b.reshape(1, K))
        in_maps = []
        for i in range(NCORES):
            rs = slice(i * R, (i + 1) * R)
            in_maps.append(
                {
                    "x0t": np.ascontiguousarray(X0[rs].T).astype(mm_np),
                    "x1t": np.ascontiguousarray(X1[rs].T).astype(mm_np),
                    "ut": np.ascontiguousarray(U[rs].T).astype(mm_np),
                    "wet": wet,
                    "at": at,
                    "ball": ball,
                    "cwt": cwt,
                    "cb": cb,
                }
            )
        return in_maps

    rows, eids = _route_slots(X0, W_enc, C_w, C_b)
    safe = np.clip(rows, 0, None)
    zero = (rows < 0)[:, None]

    def take0(M):
        out = M[safe].astype(mm_np)
        out[np.broadcast_to(zero, out.shape)] = 0
        return out

    X0s, X1s, Us = take0(X0), take0(X1), take0(U)
    wT = W_enc.T.astype(np.float32)  # [OBS, ENC]
    ae = (wT[None] @ A_all.transpose(0, 2, 1).astype(np.float32)).astype(mm_np)
    wetn = np.ascontiguousarray(-wT).astype(mm_np)
    in_maps = []
    for i in range(NCORES):
        sl = slice(i * R2, (i + 1) * R2)
        te = eids[i * NT2 : (i + 1) * NT2]
        atb = ae[te].reshape(NT2, NO, P, ENC).transpose(2, 0, 1, 3)
        in_maps.append(
            {
                "x0t": np.ascontiguousarray(X0s[sl].T),
                "x1t": np.ascontiguousarray(X1s[sl].T),
                "ut": np.ascontiguousarray(Us[sl].T),
                "wetn": wetn,
                "atb": np.ascontiguousarray(atb).reshape(P, NT2 * NO, ENC),
                "ballb": np.ascontiguousarray(ball[te].transpose(1, 0, 2)),
            }
        )
    return in_maps


def kernel(X1, X0, U, W_enc, A_all, B_rest, C_w, C_b):
    nc = _get_nc()
    in_maps = make_in_maps(X1, X0, U, W_enc, A_all, B_rest, C_w, C_b)
    res = bass_utils.run_bass_kernel_spmd(nc, in_maps, list(range(NCORES)))
    total = sum(float(r["loss_out"][0, 0]) for r in res.results)
    return np.float32(ALPHA * total / (ENC * N))


# revision 23
# speedup vs baseline: 1.0265x; 1.0265x over previous
import sys

sys.path.insert(0, "/opt/trn_rl_repo")

from contextlib import ExitStack

import ml_dtypes
import numpy as np

import concourse.bass as bass
import concourse.mybir as mybir
import concourse.tile as tile
from concourse import bacc, bass_utils

N, OBS, ENC, ACT, K = 16384, 512, 512, 64, 8
ALPHA = 1.0
NCORES = 8
R = N // NCORES  # rows per core
P = 128
NT = R // P  # n-tiles per core
NB = R // 512  # 512-wide n blocks
NH = ENC // P
NO = OBS // P
F32 = mybir.dt.float32
AX = mybir.AluOpType


def build_nc(mm_dtype=F32):
    # Bacc (not bass.Bass): its finalize() runs move_matmul_waits_to_ldweights
    # + generate_event_semaphores, required by TRN2's 1-wait-per-inst limit.
    nc = bacc.Bacc("TRN2", target_bir_lowering=False)
    x0t = nc.declare_dram_parameter("x0t", [OBS, R], mm_dtype, isOutput=False)
    x1t = nc.declare_dram_parameter("x1t", [OBS, R], mm_dtype, isOutput=False)
    ut = nc.declare_dram_parameter("ut", [ACT, R], mm_dtype, isOutput=False)
    wet = nc.declare_dram_parameter("wet", [OBS, ENC], mm_dtype, isOutput=False)
    at = nc.declare_dram_parameter("at", [K, ENC, ENC], mm_dtype, isOutput=False)
    ball = nc.declare_dram_parameter("ball", [K, ACT, ENC], mm_dtype, isOutput=False)
    cwt = nc.declare_dram_parameter("cwt", [ENC, K], mm_dtype, isOutput=False)
    cb = nc.declare_dram_parameter("cb", [1, K], F32, isOutput=False)
    loss = nc.declare_dram_parameter("loss_out", [1, 1], F32, isOutput=True)

    with tile.TileContext(nc) as tc, ExitStack() as ctx:
        const = ctx.enter_context(tc.tile_pool(name="const", bufs=1))
        stream = ctx.enter_context(tc.tile_pool(name="stream", bufs=2))
        dwork = ctx.enter_context(tc.tile_pool(name="dwork", bufs=3))
        psumA = ctx.enter_context(tc.tile_pool(name="psumA", bufs=4, space="PSUM"))
        psumS = ctx.enter_context(tc.tile_pool(name="psumS", bufs=2, space="PSUM"))

        # --- resident weights/activations ---
        wet_sb = const.tile([P, NO, ENC], mm_dtype)  # [o%128, o//128, h]
        nc.sync.dma_start(wet_sb[:], wet.rearrange("(c p) h -> p c h", p=P))
        ball_sb = const.tile([ACT, K, ENC], mm_dtype)  # [a, k, e]
        nc.sync.dma_start(ball_sb[:], ball.rearrange("k a e -> a k e"))
        cwt_sb = const.tile([P, NH, K], mm_dtype)  # [h%128, h//128, k]
        nc.sync.dma_start(cwt_sb[:], cwt.rearrange("(c p) k -> p c k", p=P))
        cb128 = const.tile([P, K], F32)
        nc.sync.dma_start(cb128[:], bass.AP(tensor=cb, offset=0, ap=[[0, P], [1, K]]))
        ut_sb = const.tile([ACT, R], mm_dtype)  # [a, n]
        nc.sync.dma_start(ut_sb[:], ut[:])

        x0et = const.tile([P, NH, R], mm_dtype)  # [h%128, h//128, n]
        x1e = const.tile([P, NT, ENC], F32)  # [n%128, n//128, e]

        iota_i = const.tile([P, K], mybir.dt.int32)
        nc.gpsimd.iota(iota_i[:], pattern=[[1, K]], base=0, channel_multiplier=0)
        iota_f = const.tile([P, K], F32)
        nc.scalar.copy(iota_f[:], iota_i[:])
        oh_all = const.tile([P, NT * K], F32)
        sq_all = const.tile([P, NT * K], F32)
        acc = const.tile([P, NT], F32)

        x0t_r = x0t.rearrange("(c p) n -> p c n", p=P)
        x1t_r = x1t.rearrange("(c p) n -> p c n", p=P)
        at_r = at.rearrange("k (c p) e -> p k c e", p=P)

        # --- phase A: encode (X0e^T and X1e) ---
        for nb in range(NB):
            ns = slice(nb * 512, (nb + 1) * 512)
            x0c = stream.tile([P, NO, 512], mm_dtype, name="x0c")
            nc.sync.dma_start(x0c[:], x0t_r[:, :, ns])
            x1c = stream.tile([P, NO, 512], mm_dtype, name="x1c")
            nc.sync.dma_start(x1c[:], x1t_r[:, :, ns])
            for hc in range(NH):
                pt = psumA.tile([P, 512], F32, name="pA")
                for oc in range(NO):
                    nc.tensor.matmul(
                        pt[:],
                        wet_sb[:, oc, hc * P : (hc + 1) * P],
                        x0c[:, oc, :],
                        start=(oc == 0),
                        stop=(oc == NO - 1),
                    )
                nc.scalar.copy(x0et[:, hc, ns], pt[:])
            for j in range(4):
                nt = nb * 4 + j
                pt = psumA.tile([P, 512], F32, name="pA")
                for oc in range(NO):
                    nc.tensor.matmul(
                        pt[:],
                        x1c[:, oc, j * P : (j + 1) * P],
                        wet_sb[:, oc, :],
                        start=(oc == 0),
                        stop=(oc == NO - 1),
                    )
                nc.scalar.copy(x1e[:, nt, :], pt[:])

        # --- phase B: router logits, argmax, one-hot ---
        for nt in range(NT):
            nts = slice(nt * P, (nt + 1) * P)
            pl = psumS.tile([P, K], F32, name="pl")
            for hc in range(NH):
                nc.tensor.matmul(
                    pl[:],
                    x0et[:, hc, nts],
                    cwt_sb[:, hc, :],
                    start=(hc == 0),
                    stop=(hc == NH - 1),
                )
            lg = dwork.tile([P, K], F32, name="lg")
            nc.vector.tensor_tensor(lg[:], pl[:], cb128[:], AX.add)
            mx = dwork.tile([P, K], F32, name="mx")
            ix = dwork.tile([P, K], mybir.dt.uint32, name="ix")
            nc.vector.max_with_indices(mx[:], ix[:], lg[:])
            ixf = dwork.tile([P, 1], F32, name="ixf")
            nc.scalar.copy(ixf[:], ix[:, 0:1])
            nc.vector.tensor_scalar(
                oh_all[:, nt * K : (nt + 1) * K],
                iota_f[:],
                ixf[:],
                None,
                op0=AX.is_equal,
            )

        # --- phase C: per-expert preds, squared error ---
        for k in range(K):
            atk = stream.tile([P, NH, ENC], mm_dtype, name="atk")
            nc.sync.dma_start(atk[:], at_r[:, k, :, :])
            for nt in range(NT):
                nts = slice(nt * P, (nt + 1) * P)
                pd = psumA.tile([P, 512], F32, name="pA")
                for hc in range(NH):
                    nc.tensor.matmul(
                        pd[:],
                        x0et[:, hc, nts],
                        atk[:, hc, :],
                        start=(hc == 0),
                        stop=False,
                    )
                nc.tensor.matmul(
                    pd[:], ut_sb[:, nts], ball_sb[:, k, :], start=False, stop=True
                )
                # GPSIMD cannot read PSUM and TensorScalarPtr is illegal on Pool:
                # vector does the subtract (PSUM->SBUF), ACT does square+accum.
                df = dwork.tile([P, ENC], F32, name="df")
                nc.vector.tensor_tensor(df[:], x1e[:, nt, :], pd[:], AX.subtract)
                sj = dwork.tile([P, ENC], F32, name="sj")
                nc.scalar.activation(
                    sj[:],
                    df[:],
                    mybir.ActivationFunctionType.Square,
                    accum_out=sq_all[:, nt * K + k : nt * K + k + 1],
                )

        # --- phase D: select routed expert's sq, accumulate ---
        for nt in range(NT):
            ks = slice(nt * K, (nt + 1) * K)
            sel = dwork.tile([P, K], F32, name="sel")
            nc.vector.scalar_tensor_tensor(
                sel[:],
                sq_all[:, ks],
                1.0,
                oh_all[:, ks],
                op0=AX.mult,
                op1=AX.mult,
                accum_out=acc[:, nt : nt + 1],
            )

        out_sb = const.tile([1, 1], F32)
        nc.gpsimd.tensor_reduce(
            out_sb[:], acc[:], axis=mybir.AxisListType.XYZWC, op=AX.add
        )
        nc.sync.dma_start(loss[:], out_sb[:])

    nc.finalize()
    return nc


T_TILES = 136  # 17408 padded slots: 16384 rows + <=127 pad/expert + global pad
NT2 = T_TILES // NCORES  # 17 tiles per core
R2 = NT2 * P  # 2176 rows per core
CHUNKS = [(0, 512), (512, 512), (1024, 512), (1536, 512), (2048, 128)]


def build_nc_routed(mm_dtype):
    # Encoder folded into weights on host: pred - x1e =
    #   x0 @ (W^T A_k^T) + u @ B_k - x1 @ W^T  -> 9 matmuls into one PSUM bank,
    # ACT squares straight from PSUM. Square kills the sign, so wetn = -W^T.
    nc = bacc.Bacc("TRN2", target_bir_lowering=False)
    x0t = nc.declare_dram_parameter("x0t", [OBS, R2], mm_dtype, isOutput=False)
    x1t = nc.declare_dram_parameter("x1t", [OBS, R2], mm_dtype, isOutput=False)
    ut = nc.declare_dram_parameter("ut", [ACT, R2], mm_dtype, isOutput=False)
    wetn = nc.declare_dram_parameter("wetn", [OBS, ENC], mm_dtype, isOutput=False)
    atb = nc.declare_dram_parameter("atb", [P, NT2 * NO, ENC], mm_dtype, isOutput=False)
    ballb = nc.declare_dram_parameter("ballb", [ACT, NT2, ENC], mm_dtype, isOutput=False)
    loss = nc.declare_dram_parameter("loss_out", [1, 1], F32, isOutput=True)

    with tile.TileContext(nc) as tc, ExitStack() as ctx:
        const = ctx.enter_context(tc.tile_pool(name="const", bufs=1))
        stream = ctx.enter_context(tc.tile_pool(name="stream", bufs=4))
        dwork = ctx.enter_context(tc.tile_pool(name="dwork", bufs=3))
        psumA = ctx.enter_context(tc.tile_pool(name="psumA", bufs=4, space="PSUM"))

        wetn_sb = const.tile([P, NO, ENC], mm_dtype)
        nc.sync.dma_start(wetn_sb[:], wetn.rearrange("(c p) h -> p c h", p=P))
        ut_sb = const.tile([ACT, R2], mm_dtype)
        nc.sync.dma_start(ut_sb[:], ut[:])
        ballb_sb = const.tile([ACT, NT2, ENC], mm_dtype)
        nc.sync.dma_start(ballb_sb[:], ballb[:])

        x0t_r = x0t.rearrange("(c p) n -> p c n", p=P)
        x1t_r = x1t.rearrange("(c p) n -> p c n", p=P)
        x0t_sb = const.tile([P, NO, R2], mm_dtype)
        x1t_sb = const.tile([P, NO, R2], mm_dtype)
        H = R2 // 2
        for h in range(2):
            hs = slice(h * H, (h + 1) * H)
            nc.sync.dma_start(x0t_sb[:, :, hs], x0t_r[:, :, hs])
            nc.sync.dma_start(x1t_sb[:, :, hs], x1t_r[:, :, hs])

        acc = const.tile([P, NT2], F32)

        for nt in range(NT2):
            nts = slice(nt * P, (nt + 1) * P)
            atk = stream.tile([P, NO, ENC], mm_dtype, name="atk")
            nc.sync.dma_start(atk[:], atb[:, nt * NO : (nt + 1) * NO, :])
            pd = psumA.tile([P, ENC], F32, name="pA")
            for oc in range(NO):
                nc.tensor.matmul(
                    pd[:],
                    x0t_sb[:, oc, nts],
                    atk[:, oc, :],
                    start=(oc == 0),
                    stop=False,
                )
            nc.tensor.matmul(
                pd[:],
                ut_sb[:, nts],
                ballb_sb[:, nt, :],
                start=False,
                stop=False,
            )
            for oc in range(NO):
                nc.tensor.matmul(
                    pd[:],
                    x1t_sb[:, oc, nts],
                    wetn_sb[:, oc, :],
                    start=False,
                    stop=(oc == NO - 1),
                )
            sj = dwork.tile([P, ENC], F32, name="sj")
            nc.scalar.activation(
                sj[:],
                pd[:],
                mybir.ActivationFunctionType.Square,
                accum_out=acc[:, nt : nt + 1],
            )

        out_sb = const.tile([1, 1], F32)
        nc.gpsimd.tensor_reduce(
            out_sb[:], acc[:], axis=mybir.AxisListType.XYZWC, op=AX.add
        )
        nc.sync.dma_start(loss[:], out_sb[:])

    nc.finalize()
    return nc


_NC_CACHE = {}
MM_BF16 = True
ROUTED = True


def _get_nc():
    key = ("routed" if ROUTED else "dense", MM_BF16)
    if key not in _NC_CACHE:
        # bf16: 1 cyc/row on PE (f32r measured ~2 due to 4B SBUF moving-read cap)
        dt = mybir.dt.bfloat16 if MM_BF16 else mybir.dt.float32r
        _NC_CACHE[key] = build_nc_routed(dt) if ROUTED else build_nc(dt)
    return _NC_CACHE[key]


def _route_slots(X0, W_enc, C_w, C_b):
    # f64 router on host: argmax(X0 @ W_enc.T @ C_w.T + C_b) per row
    m = (C_w.astype(np.float64) @ W_enc.astype(np.float64)).T  # [OBS, K]
    logits = X0.astype(np.float64) @ m + C_b.astype(np.float64)
    inds = np.argmax(logits, axis=1)
    rows_l, eids = [], []
    for k in range(K):
        rk = np.nonzero(inds == k)[0]
        pad = (-len(rk)) % P
        rows_l.append(rk)
        rows_l.append(np.full(pad, -1, np.int64))
        eids += [k] * ((len(rk) + pad) // P)
    rows = np.concatenate(rows_l)
    rows = np.concatenate([rows, np.full(T_TILES * P - len(rows), -1, np.int64)])
    eids += [0] * (T_TILES - len(eids))
    return rows, np.asarray(eids)


def make_in_maps(X1, X0, U, W_enc, A_all, B_rest, C_w, C_b):
    mm_np = ml_dtypes.bfloat16 if MM_BF16 else np.float32
    wet = np.ascontiguousarray(W_enc.T).astype(mm_np)  # [OBS, ENC]
    at = A_all.transpose(0, 2, 1).astype(mm_np)  # [K, h, e]
    b0 = np.eye(ENC, dtype=np.float32)[:ACT]
    ball = np.concatenate([b0[None], B_rest], axis=0).astype(mm_np)  # [K, a, e]

    if not ROUTED:
        cwt = np.ascontiguousarray(C_w.T).astype(mm_np)
        cb = np.ascontiguousarray(C_b.reshape(1, K))
        in_maps = []
        for i in range(NCORES):
            rs = slice(i * R, (i + 1) * R)
            in_maps.append(
                {
                    "x0t": np.ascontiguousarray(X0[rs].T).astype(mm_np),
                    "x1t": np.ascontiguousarray(X1[rs].T).astype(mm_np),
                    "ut": np.ascontiguousarray(U[rs].T).astype(mm_np),
                    "wet": wet,
                    "at": at,
                    "ball": ball,
                    "cwt": cwt,
                    "cb": cb,
                }
            )
        return in_maps

    rows, eids = _route_slots(X0, W_enc, C_w, C_b)
    safe = np.clip(rows, 0, None)
    zero = (rows < 0)[:, None]

    def take0(M):
        out = M[safe].astype(mm_np)
        out[np.broadcast_to(zero, out.shape)] = 0
        return out

    X0s, X1s, Us = take0(X0), take0(X1), take0(U)
    wT = W_enc.T.astype(np.float32)  # [OBS, ENC]
    ae = (wT[None] @ A_all.transpose(0, 2, 1).astype(np.float32)).astype(mm_np)
    wetn = np.ascontiguousarray(-wT).astype(mm_np)
    in_maps = []
    for i in range(NCORES):
        sl = slice(i * R2, (i + 1) * R2)
        te = eids[i * NT2 : (i + 1) * NT2]
        atb = ae[te].reshape(NT2, NO, P, ENC).transpose(2, 0, 1, 3)
        in_maps.append(
            {
                "x0t": np.ascontiguousarray(X0s[sl].T),
                "x1t": np.ascontiguousarray(X1s[sl].T),
                "ut": np.ascontiguousarray(Us[sl].T),
                "wetn": wetn,
                "atb": np.ascontiguousarray(atb).reshape(P, NT2 * NO, ENC),
                "ballb": np.ascontiguousarray(ball[te].transpose(1, 0, 2)),
            }
        )
    return in_maps


def kernel(X1, X0, U, W_enc, A_all, B_rest, C_w, C_b):
    nc = _get_nc()
    in_maps = make_in_maps(X1, X0, U, W_enc, A_all, B_rest, C_w, C_b)
    res = bass_utils.run_bass_kernel_spmd(nc, in_maps, list(range(NCORES)))
    total = sum(float(r["loss_out"][0, 0]) for r in res.results)
    return np.float32(ALPHA * total / (ENC * N))


# revision 24
# speedup vs baseline: 1.0320x; 1.0053x over previous
import sys

sys.path.insert(0, "/opt/trn_rl_repo")

from contextlib import ExitStack

import ml_dtypes
import numpy as np

import concourse.bass as bass
import concourse.mybir as mybir
import concourse.tile as tile
from concourse import bacc, bass_utils

N, OBS, ENC, ACT, K = 16384, 512, 512, 64, 8
ALPHA = 1.0
NCORES = 8
R = N // NCORES  # rows per core
P = 128
NT = R // P  # n-tiles per core
NB = R // 512  # 512-wide n blocks
NH = ENC // P
NO = OBS // P
F32 = mybir.dt.float32
AX = mybir.AluOpType


def build_nc(mm_dtype=F32):
    # Bacc (not bass.Bass): its finalize() runs move_matmul_waits_to_ldweights
    # + generate_event_semaphores, required by TRN2's 1-wait-per-inst limit.
    nc = bacc.Bacc("TRN2", target_bir_lowering=False)
    x0t = nc.declare_dram_parameter("x0t", [OBS, R], mm_dtype, isOutput=False)
    x1t = nc.declare_dram_parameter("x1t", [OBS, R], mm_dtype, isOutput=False)
    ut = nc.declare_dram_parameter("ut", [ACT, R], mm_dtype, isOutput=False)
    wet = nc.declare_dram_parameter("wet", [OBS, ENC], mm_dtype, isOutput=False)
    at = nc.declare_dram_parameter("at", [K, ENC, ENC], mm_dtype, isOutput=False)
    ball = nc.declare_dram_parameter("ball", [K, ACT, ENC], mm_dtype, isOutput=False)
    cwt = nc.declare_dram_parameter("cwt", [ENC, K], mm_dtype, isOutput=False)
    cb = nc.declare_dram_parameter("cb", [1, K], F32, isOutput=False)
    loss = nc.declare_dram_parameter("loss_out", [1, 1], F32, isOutput=True)

    with tile.TileContext(nc) as tc, ExitStack() as ctx:
        const = ctx.enter_context(tc.tile_pool(name="const", bufs=1))
        stream = ctx.enter_context(tc.tile_pool(name="stream", bufs=2))
        dwork = ctx.enter_context(tc.tile_pool(name="dwork", bufs=3))
        psumA = ctx.enter_context(tc.tile_pool(name="psumA", bufs=4, space="PSUM"))
        psumS = ctx.enter_context(tc.tile_pool(name="psumS", bufs=2, space="PSUM"))

        # --- resident weights/activations ---
        wet_sb = const.tile([P, NO, ENC], mm_dtype)  # [o%128, o//128, h]
        nc.sync.dma_start(wet_sb[:], wet.rearrange("(c p) h -> p c h", p=P))
        ball_sb = const.tile([ACT, K, ENC], mm_dtype)  # [a, k, e]
        nc.sync.dma_start(ball_sb[:], ball.rearrange("k a e -> a k e"))
        cwt_sb = const.tile([P, NH, K], mm_dtype)  # [h%128, h//128, k]
        nc.sync.dma_start(cwt_sb[:], cwt.rearrange("(c p) k -> p c k", p=P))
        cb128 = const.tile([P, K], F32)
        nc.sync.dma_start(cb128[:], bass.AP(tensor=cb, offset=0, ap=[[0, P], [1, K]]))
        ut_sb = const.tile([ACT, R], mm_dtype)  # [a, n]
        nc.sync.dma_start(ut_sb[:], ut[:])

        x0et = const.tile([P, NH, R], mm_dtype)  # [h%128, h//128, n]
        x1e = const.tile([P, NT, ENC], F32)  # [n%128, n//128, e]

        iota_i = const.tile([P, K], mybir.dt.int32)
        nc.gpsimd.iota(iota_i[:], pattern=[[1, K]], base=0, channel_multiplier=0)
        iota_f = const.tile([P, K], F32)
        nc.scalar.copy(iota_f[:], iota_i[:])
        oh_all = const.tile([P, NT * K], F32)
        sq_all = const.tile([P, NT * K], F32)
        acc = const.tile([P, NT], F32)

        x0t_r = x0t.rearrange("(c p) n -> p c n", p=P)
        x1t_r = x1t.rearrange("(c p) n -> p c n", p=P)
        at_r = at.rearrange("k (c p) e -> p k c e", p=P)

        # --- phase A: encode (X0e^T and X1e) ---
        for nb in range(NB):
            ns = slice(nb * 512, (nb + 1) * 512)
            x0c = stream.tile([P, NO, 512], mm_dtype, name="x0c")
            nc.sync.dma_start(x0c[:], x0t_r[:, :, ns])
            x1c = stream.tile([P, NO, 512], mm_dtype, name="x1c")
            nc.sync.dma_start(x1c[:], x1t_r[:, :, ns])
            for hc in range(NH):
                pt = psumA.tile([P, 512], F32, name="pA")
                for oc in range(NO):
                    nc.tensor.matmul(
                        pt[:],
                        wet_sb[:, oc, hc * P : (hc + 1) * P],
                        x0c[:, oc, :],
                        start=(oc == 0),
                        stop=(oc == NO - 1),
                    )
                nc.scalar.copy(x0et[:, hc, ns], pt[:])
            for j in range(4):
                nt = nb * 4 + j
                pt = psumA.tile([P, 512], F32, name="pA")
                for oc in range(NO):
                    nc.tensor.matmul(
                        pt[:],
                        x1c[:, oc, j * P : (j + 1) * P],
                        wet_sb[:, oc, :],
                        start=(oc == 0),
                        stop=(oc == NO - 1),
                    )
                nc.scalar.copy(x1e[:, nt, :], pt[:])

        # --- phase B: router logits, argmax, one-hot ---
        for nt in range(NT):
            nts = slice(nt * P, (nt + 1) * P)
            pl = psumS.tile([P, K], F32, name="pl")
            for hc in range(NH):
                nc.tensor.matmul(
                    pl[:],
                    x0et[:, hc, nts],
                    cwt_sb[:, hc, :],
                    start=(hc == 0),
                    stop=(hc == NH - 1),
                )
            lg = dwork.tile([P, K], F32, name="lg")
            nc.vector.tensor_tensor(lg[:], pl[:], cb128[:], AX.add)
            mx = dwork.tile([P, K], F32, name="mx")
            ix = dwork.tile([P, K], mybir.dt.uint32, name="ix")
            nc.vector.max_with_indices(mx[:], ix[:], lg[:])
            ixf = dwork.tile([P, 1], F32, name="ixf")
            nc.scalar.copy(ixf[:], ix[:, 0:1])
            nc.vector.tensor_scalar(
                oh_all[:, nt * K : (nt + 1) * K],
                iota_f[:],
                ixf[:],
                None,
                op0=AX.is_equal,
            )

        # --- phase C: per-expert preds, squared error ---
        for k in range(K):
            atk = stream.tile([P, NH, ENC], mm_dtype, name="atk")
            nc.sync.dma_start(atk[:], at_r[:, k, :, :])
            for nt in range(NT):
                nts = slice(nt * P, (nt + 1) * P)
                pd = psumA.tile([P, 512], F32, name="pA")
                for hc in range(NH):
                    nc.tensor.matmul(
                        pd[:],
                        x0et[:, hc, nts],
                        atk[:, hc, :],
                        start=(hc == 0),
                        stop=False,
                    )
                nc.tensor.matmul(
                    pd[:], ut_sb[:, nts], ball_sb[:, k, :], start=False, stop=True
                )
                # GPSIMD cannot read PSUM and TensorScalarPtr is illegal on Pool:
                # vector does the subtract (PSUM->SBUF), ACT does square+accum.
                df = dwork.tile([P, ENC], F32, name="df")
                nc.vector.tensor_tensor(df[:], x1e[:, nt, :], pd[:], AX.subtract)
                sj = dwork.tile([P, ENC], F32, name="sj")
                nc.scalar.activation(
                    sj[:],
                    df[:],
                    mybir.ActivationFunctionType.Square,
                    accum_out=sq_all[:, nt * K + k : nt * K + k + 1],
                )

        # --- phase D: select routed expert's sq, accumulate ---
        for nt in range(NT):
            ks = slice(nt * K, (nt + 1) * K)
            sel = dwork.tile([P, K], F32, name="sel")
            nc.vector.scalar_tensor_tensor(
                sel[:],
                sq_all[:, ks],
                1.0,
                oh_all[:, ks],
                op0=AX.mult,
                op1=AX.mult,
                accum_out=acc[:, nt : nt + 1],
            )

        out_sb = const.tile([1, 1], F32)
        nc.gpsimd.tensor_reduce(
            out_sb[:], acc[:], axis=mybir.AxisListType.XYZWC, op=AX.add
        )
        nc.sync.dma_start(loss[:], out_sb[:])

    nc.finalize()
    return nc


T_TILES = 136  # 17408 padded slots: 16384 rows + <=127 pad/expert + global pad
NT2 = T_TILES // NCORES  # 17 tiles per core
R2 = NT2 * P  # 2176 rows per core
CHUNKS = [(0, 512), (512, 512), (1024, 512), (1536, 512), (2048, 128)]


def build_nc_routed(mm_dtype):
    # Encoder folded into weights on host: pred - x1e =
    #   x0 @ (W^T A_k^T) + u @ B_k - x1 @ W^T  -> 9 matmuls into one PSUM bank,
    # ACT squares straight from PSUM. Square kills the sign, so wetn = -W^T.
    nc = bacc.Bacc("TRN2", target_bir_lowering=False)
    x0t = nc.declare_dram_parameter("x0t", [OBS, R2], mm_dtype, isOutput=False)
    x1t = nc.declare_dram_parameter("x1t", [OBS, R2], mm_dtype, isOutput=False)
    ut = nc.declare_dram_parameter("ut", [ACT, R2], mm_dtype, isOutput=False)
    wetn = nc.declare_dram_parameter("wetn", [OBS, ENC], mm_dtype, isOutput=False)
    atb = nc.declare_dram_parameter("atb", [P, NT2 * NO, ENC], mm_dtype, isOutput=False)
    ballb = nc.declare_dram_parameter("ballb", [ACT, NT2, ENC], mm_dtype, isOutput=False)
    loss = nc.declare_dram_parameter("loss_out", [1, 1], F32, isOutput=True)

    with tile.TileContext(nc) as tc, ExitStack() as ctx:
        const = ctx.enter_context(tc.tile_pool(name="const", bufs=1))
        stream = ctx.enter_context(tc.tile_pool(name="stream", bufs=NT2))
        dwork = ctx.enter_context(tc.tile_pool(name="dwork", bufs=3))
        psumA = ctx.enter_context(tc.tile_pool(name="psumA", bufs=4, space="PSUM"))

        wetn_sb = const.tile([P, NO, ENC], mm_dtype)
        nc.sync.dma_start(wetn_sb[:], wetn.rearrange("(c p) h -> p c h", p=P))
        ut_sb = const.tile([ACT, R2], mm_dtype)
        nc.sync.dma_start(ut_sb[:], ut[:])
        ballb_sb = const.tile([ACT, NT2, ENC], mm_dtype)
        nc.sync.dma_start(ballb_sb[:], ballb[:])

        x0t_r = x0t.rearrange("(c p) n -> p c n", p=P)
        x1t_r = x1t.rearrange("(c p) n -> p c n", p=P)
        x0t_sb = const.tile([P, NO, R2], mm_dtype)
        x1t_sb = const.tile([P, NO, R2], mm_dtype)
        H = R2 // 2
        for h in range(2):
            hs = slice(h * H, (h + 1) * H)
            nc.sync.dma_start(x0t_sb[:, :, hs], x0t_r[:, :, hs])
            nc.sync.dma_start(x1t_sb[:, :, hs], x1t_r[:, :, hs])

        acc = const.tile([P, NT2], F32)

        for nt in range(NT2):
            nts = slice(nt * P, (nt + 1) * P)
            atk = stream.tile([P, NO, ENC], mm_dtype, name="atk")
            nc.sync.dma_start(atk[:], atb[:, nt * NO : (nt + 1) * NO, :])
            pd = psumA.tile([P, ENC], F32, name="pA")
            for oc in range(NO):
                nc.tensor.matmul(
                    pd[:],
                    x0t_sb[:, oc, nts],
                    atk[:, oc, :],
                    start=(oc == 0),
                    stop=False,
                )
            nc.tensor.matmul(
                pd[:],
                ut_sb[:, nts],
                ballb_sb[:, nt, :],
                start=False,
                stop=False,
            )
            for oc in range(NO):
                nc.tensor.matmul(
                    pd[:],
                    x1t_sb[:, oc, nts],
                    wetn_sb[:, oc, :],
                    start=False,
                    stop=(oc == NO - 1),
                )
            sj = dwork.tile([P, ENC], F32, name="sj")
            nc.scalar.activation(
                sj[:],
                pd[:],
                mybir.ActivationFunctionType.Square,
                accum_out=acc[:, nt : nt + 1],
            )

        out_sb = const.tile([1, 1], F32)
        nc.gpsimd.tensor_reduce(
            out_sb[:], acc[:], axis=mybir.AxisListType.XYZWC, op=AX.add
        )
        nc.sync.dma_start(loss[:], out_sb[:])

    nc.finalize()
    return nc


_NC_CACHE = {}
MM_BF16 = True
ROUTED = True


def _get_nc():
    key = ("routed" if ROUTED else "dense", MM_BF16)
    if key not in _NC_CACHE:
        # bf16: 1 cyc/row on PE (f32r measured ~2 due to 4B SBUF moving-read cap)
        dt = mybir.dt.bfloat16 if MM_BF16 else mybir.dt.float32r
        _NC_CACHE[key] = build_nc_routed(dt) if ROUTED else build_nc(dt)
    return _NC_CACHE[key]


def _route_slots(X0, W_enc, C_w, C_b):
    # f64 router on host: argmax(X0 @ W_enc.T @ C_w.T + C_b) per row
    m = (C_w.astype(np.float64) @ W_enc.astype(np.float64)).T  # [OBS, K]
    logits = X0.astype(np.float64) @ m + C_b.astype(np.float64)
    inds = np.argmax(logits, axis=1)
    rows_l, eids = [], []
    for k in range(K):
        rk = np.nonzero(inds == k)[0]
        pad = (-len(rk)) % P
        rows_l.append(rk)
        rows_l.append(np.full(pad, -1, np.int64))
        eids += [k] * ((len(rk) + pad) // P)
    rows = np.concatenate(rows_l)
    rows = np.concatenate([rows, np.full(T_TILES * P - len(rows), -1, np.int64)])
    eids += [0] * (T_TILES - len(eids))
    return rows, np.asarray(eids)


def make_in_maps(X1, X0, U, W_enc, A_all, B_rest, C_w, C_b):
    mm_np = ml_dtypes.bfloat16 if MM_BF16 else np.float32
    wet = np.ascontiguousarray(W_enc.T).astype(mm_np)  # [OBS, ENC]
    at = A_all.transpose(0, 2, 1).astype(mm_np)  # [K, h, e]
    b0 = np.eye(ENC, dtype=np.float32)[:ACT]
    ball = np.concatenate([b0[None], B_rest], axis=0).astype(mm_np)  # [K, a, e]

    if not ROUTED:
        cwt = np.ascontiguousarray(C_w.T).astype(mm_np)
        cb = np.ascontiguousarray(C_b.reshape(1, K))
        in_maps = []
        for i in range(NCORES):
            rs = slice(i * R, (i + 1) * R)
            in_maps.append(
                {
                    "x0t": np.ascontiguousarray(X0[rs].T).astype(mm_np),
                    "x1t": np.ascontiguousarray(X1[rs].T).astype(mm_np),
                    "ut": np.ascontiguousarray(U[rs].T).astype(mm_np),
                    "wet": wet,
                    "at": at,
                    "ball": ball,
                    "cwt": cwt,
                    "cb": cb,
                }
            )
        return in_maps

    rows, eids = _route_slots(X0, W_enc, C_w, C_b)
    safe = np.clip(rows, 0, None)
    zero = (rows < 0)[:, None]

    def take0(M):
        out = M[safe].astype(mm_np)
        out[np.broadcast_to(zero, out.shape)] = 0
        return out

    X0s, X1s, Us = take0(X0), take0(X1), take0(U)
    wT = W_enc.T.astype(np.float32)  # [OBS, ENC]
    ae = (wT[None] @ A_all.transpose(0, 2, 1).astype(np.float32)).astype(mm_np)
    wetn = np.ascontiguousarray(-wT).astype(mm_np)
    in_maps = []
    for i in range(NCORES):
        sl = slice(i * R2, (i + 1) * R2)
        te = eids[i * NT2 : (i + 1) * NT2]
        atb = ae[te].reshape(NT2, NO, P, ENC).transpose(2, 0, 1, 3)
        in_maps.append(
            {
                "x0t": np.ascontiguousarray(X0s[sl].T),
                "x1t": np.ascontiguousarray(X1s[sl].T),
                "ut": np.ascontiguousarray(Us[sl].T),
                "wetn": wetn,
                "atb": np.ascontiguousarray(atb).reshape(P, NT2 * NO, ENC),
                "ballb": np.ascontiguousarray(ball[te].transpose(1, 0, 2)),
            }
        )
    return in_maps


def kernel(X1, X0, U, W_enc, A_all, B_rest, C_w, C_b):
    nc = _get_nc()
    in_maps = make_in_maps(X1, X0, U, W_enc, A_all, B_rest, C_w, C_b)
    res = bass_utils.run_bass_kernel_spmd(nc, in_maps, list(range(NCORES)))
    total = sum(float(r["loss_out"][0, 0]) for r in res.results)
    return np.float32(ALPHA * total / (ENC * N))


# revision 25
# speedup vs baseline: 1.0387x; 1.0065x over previous
import sys

sys.path.insert(0, "/opt/trn_rl_repo")

from contextlib import ExitStack

import ml_dtypes
import numpy as np

import concourse.bass as bass
import concourse.mybir as mybir
import concourse.tile as tile
from concourse import bacc, bass_utils

N, OBS, ENC, ACT, K = 16384, 512, 512, 64, 8
ALPHA = 1.0
NCORES = 8
R = N // NCORES  # rows per core
P = 128
NT = R // P  # n-tiles per core
NB = R // 512  # 512-wide n blocks
NH = ENC // P
NO = OBS // P
F32 = mybir.dt.float32
AX = mybir.AluOpType


def build_nc(mm_dtype=F32):
    # Bacc (not bass.Bass): its finalize() runs move_matmul_waits_to_ldweights
    # + generate_event_semaphores, required by TRN2's 1-wait-per-inst limit.
    nc = bacc.Bacc("TRN2", target_bir_lowering=False)
    x0t = nc.declare_dram_parameter("x0t", [OBS, R], mm_dtype, isOutput=False)
    x1t = nc.declare_dram_parameter("x1t", [OBS, R], mm_dtype, isOutput=False)
    ut = nc.declare_dram_parameter("ut", [ACT, R], mm_dtype, isOutput=False)
    wet = nc.declare_dram_parameter("wet", [OBS, ENC], mm_dtype, isOutput=False)
    at = nc.declare_dram_parameter("at", [K, ENC, ENC], mm_dtype, isOutput=False)
    ball = nc.declare_dram_parameter("ball", [K, ACT, ENC], mm_dtype, isOutput=False)
    cwt = nc.declare_dram_parameter("cwt", [ENC, K], mm_dtype, isOutput=False)
    cb = nc.declare_dram_parameter("cb", [1, K], F32, isOutput=False)
    loss = nc.declare_dram_parameter("loss_out", [1, 1], F32, isOutput=True)

    with tile.TileContext(nc) as tc, ExitStack() as ctx:
        const = ctx.enter_context(tc.tile_pool(name="const", bufs=1))
        stream = ctx.enter_context(tc.tile_pool(name="stream", bufs=2))
        dwork = ctx.enter_context(tc.tile_pool(name="dwork", bufs=3))
        psumA = ctx.enter_context(tc.tile_pool(name="psumA", bufs=4, space="PSUM"))
        psumS = ctx.enter_context(tc.tile_pool(name="psumS", bufs=2, space="PSUM"))

        # --- resident weights/activations ---
        wet_sb = const.tile([P, NO, ENC], mm_dtype)  # [o%128, o//128, h]
        nc.sync.dma_start(wet_sb[:], wet.rearrange("(c p) h -> p c h", p=P))
        ball_sb = const.tile([ACT, K, ENC], mm_dtype)  # [a, k, e]
        nc.sync.dma_start(ball_sb[:], ball.rearrange("k a e -> a k e"))
        cwt_sb = const.tile([P, NH, K], mm_dtype)  # [h%128, h//128, k]
        nc.sync.dma_start(cwt_sb[:], cwt.rearrange("(c p) k -> p c k", p=P))
        cb128 = const.tile([P, K], F32)
        nc.sync.dma_start(cb128[:], bass.AP(tensor=cb, offset=0, ap=[[0, P], [1, K]]))
        ut_sb = const.tile([ACT, R], mm_dtype)  # [a, n]
        nc.sync.dma_start(ut_sb[:], ut[:])

        x0et = const.tile([P, NH, R], mm_dtype)  # [h%128, h//128, n]
        x1e = const.tile([P, NT, ENC], F32)  # [n%128, n//128, e]

        iota_i = const.tile([P, K], mybir.dt.int32)
        nc.gpsimd.iota(iota_i[:], pattern=[[1, K]], base=0, channel_multiplier=0)
        iota_f = const.tile([P, K], F32)
        nc.scalar.copy(iota_f[:], iota_i[:])
        oh_all = const.tile([P, NT * K], F32)
        sq_all = const.tile([P, NT * K], F32)
        acc = const.tile([P, NT], F32)

        x0t_r = x0t.rearrange("(c p) n -> p c n", p=P)
        x1t_r = x1t.rearrange("(c p) n -> p c n", p=P)
        at_r = at.rearrange("k (c p) e -> p k c e", p=P)

        # --- phase A: encode (X0e^T and X1e) ---
        for nb in range(NB):
            ns = slice(nb * 512, (nb + 1) * 512)
            x0c = stream.tile([P, NO, 512], mm_dtype, name="x0c")
            nc.sync.dma_start(x0c[:], x0t_r[:, :, ns])
            x1c = stream.tile([P, NO, 512], mm_dtype, name="x1c")
            nc.sync.dma_start(x1c[:], x1t_r[:, :, ns])
            for hc in range(NH):
                pt = psumA.tile([P, 512], F32, name="pA")
                for oc in range(NO):
                    nc.tensor.matmul(
                        pt[:],
                        wet_sb[:, oc, hc * P : (hc + 1) * P],
                        x0c[:, oc, :],
                        start=(oc == 0),
                        stop=(oc == NO - 1),
                    )
                nc.scalar.copy(x0et[:, hc, ns], pt[:])
            for j in range(4):
                nt = nb * 4 + j
                pt = psumA.tile([P, 512], F32, name="pA")
                for oc in range(NO):
                    nc.tensor.matmul(
                        pt[:],
                        x1c[:, oc, j * P : (j + 1) * P],
                        wet_sb[:, oc, :],
                        start=(oc == 0),
                        stop=(oc == NO - 1),
                    )
                nc.scalar.copy(x1e[:, nt, :], pt[:])

        # --- phase B: router logits, argmax, one-hot ---
        for nt in range(NT):
            nts = slice(nt * P, (nt + 1) * P)
            pl = psumS.tile([P, K], F32, name="pl")
            for hc in range(NH):
                nc.tensor.matmul(
                    pl[:],
                    x0et[:, hc, nts],
                    cwt_sb[:, hc, :],
                    start=(hc == 0),
                    stop=(hc == NH - 1),
                )
            lg = dwork.tile([P, K], F32, name="lg")
            nc.vector.tensor_tensor(lg[:], pl[:], cb128[:], AX.add)
            mx = dwork.tile([P, K], F32, name="mx")
            ix = dwork.tile([P, K], mybir.dt.uint32, name="ix")
            nc.vector.max_with_indices(mx[:], ix[:], lg[:])
            ixf = dwork.tile([P, 1], F32, name="ixf")
            nc.scalar.copy(ixf[:], ix[:, 0:1])
            nc.vector.tensor_scalar(
                oh_all[:, nt * K : (nt + 1) * K],
                iota_f[:],
                ixf[:],
                None,
                op0=AX.is_equal,
            )

        # --- phase C: per-expert preds, squared error ---
        for k in range(K):
            atk = stream.tile([P, NH, ENC], mm_dtype, name="atk")
            nc.sync.dma_start(atk[:], at_r[:, k, :, :])
            for nt in range(NT):
                nts = slice(nt * P, (nt + 1) * P)
                pd = psumA.tile([P, 512], F32, name="pA")
                for hc in range(NH):
                    nc.tensor.matmul(
                        pd[:],
                        x0et[:, hc, nts],
                        atk[:, hc, :],
                        start=(hc == 0),
                        stop=False,
                    )
                nc.tensor.matmul(
                    pd[:], ut_sb[:, nts], ball_sb[:, k, :], start=False, stop=True
                )
                # GPSIMD cannot read PSUM and TensorScalarPtr is illegal on Pool:
                # vector does the subtract (PSUM->SBUF), ACT does square+accum.
                df = dwork.tile([P, ENC], F32, name="df")
                nc.vector.tensor_tensor(df[:], x1e[:, nt, :], pd[:], AX.subtract)
                sj = dwork.tile([P, ENC], F32, name="sj")
                nc.scalar.activation(
                    sj[:],
                    df[:],
                    mybir.ActivationFunctionType.Square,
                    accum_out=sq_all[:, nt * K + k : nt * K + k + 1],
                )

        # --- phase D: select routed expert's sq, accumulate ---
        for nt in range(NT):
            ks = slice(nt * K, (nt + 1) * K)
            sel = dwork.tile([P, K], F32, name="sel")
            nc.vector.scalar_tensor_tensor(
                sel[:],
                sq_all[:, ks],
                1.0,
                oh_all[:, ks],
                op0=AX.mult,
                op1=AX.mult,
                accum_out=acc[:, nt : nt + 1],
            )

        out_sb = const.tile([1, 1], F32)
        nc.gpsimd.tensor_reduce(
            out_sb[:], acc[:], axis=mybir.AxisListType.XYZWC, op=AX.add
        )
        nc.sync.dma_start(loss[:], out_sb[:])

    nc.finalize()
    return nc


T_TILES = 136  # 17408 padded slots: 16384 rows + <=127 pad/expert + global pad
NT2 = T_TILES // NCORES  # 17 tiles per core
R2 = NT2 * P  # 2176 rows per core
CHUNKS = [(0, 512), (512, 512), (1024, 512), (1536, 512), (2048, 128)]


def build_nc_routed(mm_dtype):
    # Encoder folded into weights on host: pred - x1e =
    #   x0 @ (W^T A_k^T) + u @ B_k - x1 @ W^T  -> 9 matmuls into one PSUM bank,
    # ACT squares straight from PSUM. Square kills the sign, so wetn = -W^T.
    nc = bacc.Bacc("TRN2", target_bir_lowering=False)
    x0t = nc.declare_dram_parameter("x0t", [OBS, R2], mm_dtype, isOutput=False)
    x1t = nc.declare_dram_parameter("x1t", [OBS, R2], mm_dtype, isOutput=False)
    ut = nc.declare_dram_parameter("ut", [ACT, R2], mm_dtype, isOutput=False)
    wetn = nc.declare_dram_parameter("wetn", [OBS, ENC], mm_dtype, isOutput=False)
    atb = nc.declare_dram_parameter("atb", [P, NT2 * NO, ENC], mm_dtype, isOutput=False)
    ballb = nc.declare_dram_parameter("ballb", [ACT, NT2, ENC], mm_dtype, isOutput=False)
    loss = nc.declare_dram_parameter("loss_out", [1, 1], F32, isOutput=True)

    with tile.TileContext(nc) as tc, ExitStack() as ctx:
        const = ctx.enter_context(tc.tile_pool(name="const", bufs=1))
        stream = ctx.enter_context(tc.tile_pool(name="stream", bufs=NT2))
        dwork = ctx.enter_context(tc.tile_pool(name="dwork", bufs=3))
        psumA = ctx.enter_context(tc.tile_pool(name="psumA", bufs=8, space="PSUM"))

        wetn_sb = const.tile([P, NO, ENC], mm_dtype)
        nc.sync.dma_start(wetn_sb[:], wetn.rearrange("(c p) h -> p c h", p=P))
        ut_sb = const.tile([ACT, R2], mm_dtype)
        nc.sync.dma_start(ut_sb[:], ut[:])
        ballb_sb = const.tile([ACT, NT2, ENC], mm_dtype)
        nc.sync.dma_start(ballb_sb[:], ballb[:])

        x0t_r = x0t.rearrange("(c p) n -> p c n", p=P)
        x1t_r = x1t.rearrange("(c p) n -> p c n", p=P)
        x0t_sb = const.tile([P, NO, R2], mm_dtype)
        x1t_sb = const.tile([P, NO, R2], mm_dtype)
        H = R2 // 2
        for h in range(2):
            hs = slice(h * H, (h + 1) * H)
            nc.sync.dma_start(x0t_sb[:, :, hs], x0t_r[:, :, hs])
            nc.sync.dma_start(x1t_sb[:, :, hs], x1t_r[:, :, hs])

        acc = const.tile([P, NT2], F32)

        for nt in range(NT2):
            nts = slice(nt * P, (nt + 1) * P)
            atk = stream.tile([P, NO, ENC], mm_dtype, name="atk")
            nc.sync.dma_start(atk[:], atb[:, nt * NO : (nt + 1) * NO, :])
            pd = psumA.tile([P, ENC], F32, name="pA")
            for oc in range(NO):
                nc.tensor.matmul(
                    pd[:],
                    x0t_sb[:, oc, nts],
                    atk[:, oc, :],
                    start=(oc == 0),
                    stop=False,
                )
            nc.tensor.matmul(
                pd[:],
                ut_sb[:, nts],
                ballb_sb[:, nt, :],
                start=False,
                stop=False,
            )
            for oc in range(NO):
                nc.tensor.matmul(
                    pd[:],
                    x1t_sb[:, oc, nts],
                    wetn_sb[:, oc, :],
                    start=False,
                    stop=(oc == NO - 1),
                )
            sj = dwork.tile([P, ENC], F32, name="sj")
            nc.scalar.activation(
                sj[:],
                pd[:],
                mybir.ActivationFunctionType.Square,
                accum_out=acc[:, nt : nt + 1],
            )

        out_sb = const.tile([1, 1], F32)
        nc.gpsimd.tensor_reduce(
            out_sb[:], acc[:], axis=mybir.AxisListType.XYZWC, op=AX.add
        )
        nc.sync.dma_start(loss[:], out_sb[:])

    nc.finalize()
    return nc


_NC_CACHE = {}
MM_BF16 = True
ROUTED = True


def _get_nc():
    key = ("routed" if ROUTED else "dense", MM_BF16)
    if key not in _NC_CACHE:
        # bf16: 1 cyc/row on PE (f32r measured ~2 due to 4B SBUF moving-read cap)
        dt = mybir.dt.bfloat16 if MM_BF16 else mybir.dt.float32r
        _NC_CACHE[key] = build_nc_routed(dt) if ROUTED else build_nc(dt)
    return _NC_CACHE[key]


def _route_slots(X0, W_enc, C_w, C_b):
    # f64 router on host: argmax(X0 @ W_enc.T @ C_w.T + C_b) per row
    m = (C_w.astype(np.float64) @ W_enc.astype(np.float64)).T  # [OBS, K]
    logits = X0.astype(np.float64) @ m + C_b.astype(np.float64)
    inds = np.argmax(logits, axis=1)
    rows_l, eids = [], []
    for k in range(K):
        rk = np.nonzero(inds == k)[0]
        pad = (-len(rk)) % P
        rows_l.append(rk)
        rows_l.append(np.full(pad, -1, np.int64))
        eids += [k] * ((len(rk) + pad) // P)
    rows = np.concatenate(rows_l)
    rows = np.concatenate([rows, np.full(T_TILES * P - len(rows), -1, np.int64)])
    eids += [0] * (T_TILES - len(eids))
    return rows, np.asarray(eids)


def make_in_maps(X1, X0, U, W_enc, A_all, B_rest, C_w, C_b):
    mm_np = ml_dtypes.bfloat16 if MM_BF16 else np.float32
    wet = np.ascontiguousarray(W_enc.T).astype(mm_np)  # [OBS, ENC]
    at = A_all.transpose(0, 2, 1).astype(mm_np)  # [K, h, e]
    b0 = np.eye(ENC, dtype=np.float32)[:ACT]
    ball = np.concatenate([b0[None], B_rest], axis=0).astype(mm_np)  # [K, a, e]

    if not ROUTED:
        cwt = np.ascontiguousarray(C_w.T).astype(mm_np)
        cb = np.ascontiguousarray(C_b.reshape(1, K))
        in_maps = []
        for i in range(NCORES):
            rs = slice(i * R, (i + 1) * R)
            in_maps.append(
                {
                    "x0t": np.ascontiguousarray(X0[rs].T).astype(mm_np),
                    "x1t": np.ascontiguousarray(X1[rs].T).astype(mm_np),
                    "ut": np.ascontiguousarray(U[rs].T).astype(mm_np),
                    "wet": wet,
                    "at": at,
                    "ball": ball,
                    "cwt": cwt,
                    "cb": cb,
                }
            )
        return in_maps

    rows, eids = _route_slots(X0, W_enc, C_w, C_b)
    safe = np.clip(rows, 0, None)
    zero = (rows < 0)[:, None]

    def take0(M):
        out = M[safe].astype(mm_np)
        out[np.broadcast_to(zero, out.shape)] = 0
        return out

    X0s, X1s, Us = take0(X0), take0(X1), take0(U)
    wT = W_enc.T.astype(np.float32)  # [OBS, ENC]
    ae = (wT[None] @ A_all.transpose(0, 2, 1).astype(np.float32)).astype(mm_np)
    wetn = np.ascontiguousarray(-wT).astype(mm_np)
    in_maps = []
    for i in range(NCORES):
        sl = slice(i * R2, (i + 1) * R2)
        te = eids[i * NT2 : (i + 1) * NT2]
        atb = ae[te].reshape(NT2, NO, P, ENC).transpose(2, 0, 1, 3)
        in_maps.append(
            {
                "x0t": np.ascontiguousarray(X0s[sl].T),
                "x1t": np.ascontiguousarray(X1s[sl].T),
                "ut": np.ascontiguousarray(Us[sl].T),
                "wetn": wetn,
                "atb": np.ascontiguousarray(atb).reshape(P, NT2 * NO, ENC),
                "ballb": np.ascontiguousarray(ball[te].transpose(1, 0, 2)),
            }
        )
    return in_maps


def kernel(X1, X0, U, W_enc, A_all, B_rest, C_w, C_b):
    nc = _get_nc()
    in_maps = make_in_maps(X1, X0, U, W_enc, A_all, B_rest, C_w, C_b)
    res = bass_utils.run_bass_kernel_spmd(nc, in_maps, list(range(NCORES)))
    total = sum(float(r["loss_out"][0, 0]) for r in res.results)
    return np.float32(ALPHA * total / (ENC * N))
